# revision 1
# baseline (speedup 1.0000x reference)
"""GAT GNN kernel for 8 Trainium2 NeuronCores (Bass, via PJRT/axon).

Strategy (per spec sharding_hint): partition dst nodes (and their incoming
edges) across 8 cores. Nodes are permuted by in-degree so each 128-node dst
tile has near-uniform degree -> tight ELL (padded CSR) slot grids. Per tile:
  - indirect-DMA gather of [prev | es | ed] rows for every edge slot
    (slot 0 = self loop, also supplies ed[dst] per partition)
  - on-chip segment softmax: z = es[src]+ed[dst]; lrelu; per-row (=per dst)
    max/exp/sum on DVE+ACT; p = exp(lz - m)
  - aggregation: feats *= p (DVE), reduce over slots (DVE)
  - out = (agg/den) @ W + b via PE (transpose + matmul), exploiting
    (sum_e a_e prev[src]) @ W == sum_e a_e (prev@W)[src]
Three launches (L1, L2, L3); host applies relu and computes next-layer
es/ed = prev @ (W@a) between launches, then mean/max-pools by graph and
applies the final linear. L2 and L3 share one compiled kernel.
"""
import os
import sys
import math

sys.path.insert(0, "/opt/trn_rl_repo")

import numpy as np

P = 128
F_OUT = 64
NEG_SLOPE = 0.2
N_CORES = 8
COLS_BUDGET = {130: 96, 66: 160}  # gather cols per group, by row width
MAX_NT = 4

_RUNNERS = {}


def _make_runner(nc, replicated_names):
    """jit the bass module over 8 cores via shard_map; returns fn(global_ins)->
    np [8*SHR, 64]. Inputs in replicated_names get PartitionSpec(None)."""
    import jax
    from jax.sharding import Mesh, PartitionSpec
    from jax.experimental.shard_map import shard_map
    import concourse.mybir as mybir
    from concourse.bass2jax import (_bass_exec_p, partition_id_tensor,
                                    install_neuronx_cc_hook)

    install_neuronx_cc_hook()
    nc.finalize()
    partition_name = nc.partition_id_tensor.name if nc.partition_id_tensor else None

    in_names, out_names, out_avals, zero_outs = [], [], [], []
    for alloc in nc.m.functions[0].allocations:
        if not isinstance(alloc, mybir.MemoryLocationSet):
            continue
        name = alloc.memorylocations[0].name
        if alloc.kind == "ExternalInput":
            if name != partition_name:
                in_names.append(name)
        elif alloc.kind == "ExternalOutput":
            shape = tuple(alloc.tensor_shape)
            dtype = mybir.dt.np(alloc.dtype)
            out_names.append(name)
            out_avals.append(jax.core.ShapedArray(shape, dtype))
            zero_outs.append(np.zeros(shape, dtype))
    n_params = len(in_names)
    all_in = in_names + out_names + ([partition_name] if partition_name else [])

    def _body(*args):
        operands = list(args)
        if partition_name is not None:
            operands.append(partition_id_tensor())
        return tuple(_bass_exec_p.bind(
            *operands,
            out_avals=tuple(out_avals), in_names=tuple(all_in),
            out_names=tuple(out_names), lowering_input_output_aliases=(),
            sim_require_finite=False, sim_require_nnan=False, nc=nc))

    devices = jax.devices()[:N_CORES]
    mesh = Mesh(np.asarray(devices), ("core",))
    in_specs = tuple(
        PartitionSpec(None) if n in replicated_names else PartitionSpec("core")
        for n in in_names) + (PartitionSpec("core"),) * len(out_names)
    out_specs = (PartitionSpec("core"),) * len(out_names)
    jfn = jax.jit(shard_map(_body, mesh=mesh, in_specs=in_specs,
                            out_specs=out_specs, check_rep=False),
                  keep_unused=True)

    def fn(global_ins):
        args = [global_ins[n] for n in in_names]
        args += [np.zeros((N_CORES * z.shape[0], *z.shape[1:]), z.dtype)
                 for z in zero_outs]
        outs = jfn(*args)
        jax.block_until_ready(outs)
        return np.asarray(outs[0])

    return fn, in_names


def _build_layer_kernel(RC, R_TOT, groups, totcols, shr_rows):
    """One GAT layer for one core's dst shard.

    RC: gathered row width (K_IN feats + es + ed). groups: list of
    (col_off, row_off, nt, Kg). Output: [shr_rows, 64] raw (no relu)."""
    import concourse.bacc as bacc
    import concourse.bass as bass
    import concourse.mybir as mybir
    import concourse.tile as tile
    from concourse.masks import make_identity

    DT = mybir.dt.float32
    A = mybir.AluOpType
    K_IN = RC - 2
    nc = bacc.Bacc("TRN2", target_bir_lowering=False, debug=False,
                   num_devices=N_CORES)
    tbl = nc.dram_tensor("tbl", [R_TOT, RC], DT, kind="ExternalInput")
    idx = nc.dram_tensor("idx", [P, totcols], mybir.dt.uint32,
                         kind="ExternalInput")
    W_d = nc.dram_tensor("w", [K_IN, F_OUT], DT, kind="ExternalInput")
    b_d = nc.dram_tensor("b", [P, F_OUT], DT, kind="ExternalInput")
    out_d = nc.dram_tensor("out", [shr_rows, F_OUT], DT, kind="ExternalOutput")

    with tile.TileContext(nc) as tc:
        with (tc.tile_pool(name="const", bufs=1) as cpool,
              tc.tile_pool(name="sb", bufs=2) as pool,
              tc.tile_pool(name="ps", bufs=2, space="PSUM") as pspool):
            ident = cpool.tile([P, P], DT)
            make_identity(nc, ident[:])
            w_sb = cpool.tile([K_IN, F_OUT], DT)
            nc.sync.dma_start(out=w_sb[:], in_=W_d[:])
            b_sb = cpool.tile([P, F_OUT], DT)
            nc.sync.dma_start(out=b_sb[:], in_=b_d[:])

            for (col_off, row_off, nt, Kg) in groups:
                cols = nt * Kg
                it = pool.tile([P, cols], mybir.dt.uint32, tag="idx")
                nc.sync.dma_start(out=it[:], in_=idx[:, col_off:col_off + cols])
                g = pool.tile([P, cols * RC], DT, tag="g")
                for cc in range(cols):
                    nc.gpsimd.indirect_dma_start(
                        out=g[:, cc * RC:(cc + 1) * RC], out_offset=None,
                        in_=tbl[:],
                        in_offset=bass.IndirectOffsetOnAxis(
                            ap=it[:, cc:cc + 1], axis=0))
                gb = g[:]
                pstep = gb.ap[0][0]

                def gap(off, dims):
                    return bass.AP(gb.tensor, gb.offset + off,
                                   [[pstep, P]] + dims)

                # z = es_slot + ed_own  (ed from self-loop slot 0 per tile)
                z = pool.tile([P, cols], DT, tag="z")
                nc.vector.tensor_tensor(
                    out=z[:],
                    in0=gap(K_IN, [[RC, cols]]),
                    in1=gap(K_IN + 1, [[Kg * RC, nt], [0, Kg]]),
                    op=A.add)
                # leaky relu (in place, exact): z = max(max(z, 0.2z), -30)
                zt = pool.tile([P, cols], DT, tag="zt")
                nc.vector.tensor_scalar_mul(zt[:], z[:], NEG_SLOPE)
                nc.vector.tensor_tensor(out=z[:], in0=z[:], in1=zt[:], op=A.max)
                nc.vector.tensor_scalar_max(z[:], z[:], -30.0)
                zv = z[:].rearrange("p (t k) -> p t k", k=Kg)
                nc.scalar.activation(z[:], z[:],
                                     mybir.ActivationFunctionType.Exp)
                # den and 1/den
                den = pool.tile([P, nt], DT, tag="den")
                nc.vector.tensor_reduce(out=den[:], in_=zv,
                                        axis=mybir.AxisListType.X, op=A.add)
                nc.vector.reciprocal(den[:], den[:])
                # feats *= p  (in place on gathered rows)
                zb = z[:]
                nc.vector.tensor_tensor(
                    out=gap(0, [[RC, cols], [1, K_IN]]),
                    in0=gap(0, [[RC, cols], [1, K_IN]]),
                    in1=bass.AP(zb.tensor, zb.offset,
                                [[zb.ap[0][0], P], [1, cols], [0, K_IN]]),
                    op=A.mult)
                # reduce over slots -> agg [P, nt*K_IN]
                agg = pool.tile([P, nt * K_IN], DT, tag="agg")
                nc.vector.tensor_reduce(
                    out=agg[:],
                    in_=gap(0, [[Kg * RC, nt], [1, K_IN], [RC, Kg]]),
                    axis=mybir.AxisListType.X, op=A.add)
                # agg *= 1/den
                db = den[:]
                nc.vector.tensor_tensor(
                    out=agg[:], in0=agg[:],
                    in1=bass.AP(db.tensor, db.offset,
                                [[db.ap[0][0], P], [1, nt], [0, K_IN]]),
                    op=A.mult)
                # transpose each tile's agg, then matmul with W
                psT = pspool.tile([K_IN, nt * P], DT, tag="psT")
                aggv = agg[:].rearrange("p (t f) -> p t f", f=K_IN)
                for t in range(nt):
                    nc.tensor.transpose(out=psT[:, t * P:(t + 1) * P],
                                        in_=aggv[:, t, :], identity=ident[:])
                aggT = pool.tile([K_IN, nt * P], DT, tag="aggT")
                nc.vector.tensor_copy(out=aggT[:], in_=psT[:])
                psO = pspool.tile([P, nt * F_OUT], DT, tag="psO")
                for t in range(nt):
                    nc.tensor.matmul(out=psO[:, t * F_OUT:(t + 1) * F_OUT],
                                     lhsT=aggT[:, t * P:(t + 1) * P],
                                     rhs=w_sb[:], start=True, stop=True)
                outt = pool.tile([P, nt * F_OUT], DT, tag="outt")
                bb = b_sb[:]
                nc.vector.tensor_tensor(
                    out=outt[:], in0=psO[:],
                    in1=bass.AP(bb.tensor, bb.offset,
                                [[bb.ap[0][0], P], [0, nt], [1, F_OUT]]),
                    op=A.add)
                # write rows: row (t, p) -> shard row row_off + t*128 + p
                ob = out_d[:]
                dst_ap = bass.AP(ob.tensor, ob.offset + row_off * F_OUT,
                                 [[F_OUT, P], [P * F_OUT, nt], [1, F_OUT]])
                nc.sync.dma_start(out=dst_ap, in_=outt[:])
    return nc


def _prep_graph(N, src, dst):
    """Degree-permuted ELL layout. Returns dict with ranks, tiles, groups,
    and per-core idx arrays."""
    deg = np.bincount(dst, minlength=N).astype(np.int64) + 1  # + self loop
    order = np.argsort(deg, kind="stable")     # node id per rank
    rank = np.empty(N, np.int64)
    rank[order] = np.arange(N)
    n_tiles = (N + P - 1) // P
    R_TOT = (n_tiles + 1) * P
    DUMMY = R_TOT - 1

    # CSR over dst ranks
    dstr = rank[dst]
    srcr = rank[src].astype(np.uint32)
    ord_e = np.argsort(dstr, kind="stable")
    dstr_s = dstr[ord_e]
    srcr_s = srcr[ord_e]
    indptr = np.searchsorted(dstr_s, np.arange(N + 1))

    # per-rank degree (incl self), padded ranks get 0 slots (all dummy)
    degr = np.zeros(R_TOT, np.int64)
    degr[:N] = deg[order]

    T_core = (n_tiles + N_CORES - 1) // N_CORES
    # K per tile position j (max over cores, tiles j*8+c), >=2
    K_hat = np.zeros(T_core, np.int64)
    for j in range(T_core):
        ts = [j * N_CORES + c for c in range(N_CORES) if j * N_CORES + c < n_tiles]
        K_hat[j] = max(2, max(int(degr[t * P:(t + 1) * P].max()) for t in ts))

    budget = None  # set by caller per RC
    return dict(order=order, rank=rank, n_tiles=n_tiles, R_TOT=R_TOT,
                DUMMY=DUMMY, srcr_s=srcr_s, indptr=indptr, degr=degr,
                T_core=T_core, K_hat=K_hat)


def _make_groups(K_hat, budget):
    groups = []
    j = 0
    T = len(K_hat)
    col_off = 0
    while j < T:
        nt = 1
        kg = int(K_hat[j])
        while (j + nt < T and nt < MAX_NT
               and (nt + 1) * max(kg, int(K_hat[j + nt])) <= budget):
            kg = max(kg, int(K_hat[j + nt]))
            nt += 1
        groups.append((col_off, j * P, nt, kg))
        col_off += nt * kg
        j += nt
    return groups, col_off


def _fill_idx(gp, groups, totcols):
    """Per-core idx arrays [P, totcols] uint32 (slot 0 = self rank)."""
    srcr_s, indptr, degr = gp["srcr_s"], gp["indptr"], gp["degr"]
    n_tiles, DUMMY, T_core = gp["n_tiles"], gp["DUMMY"], gp["T_core"]
    N = len(indptr) - 1
    idxs = np.full((N_CORES, P, totcols), DUMMY, np.uint32)
    for c in range(N_CORES):
        for (col_off, row_off, nt, Kg) in groups:
            for t in range(nt):
                j = row_off // P + t
                tile_id = j * N_CORES + c
                if tile_id >= n_tiles:
                    continue
                r0 = tile_id * P
                ranks = np.arange(r0, r0 + P)
                real = ranks < N
                co = col_off + t * Kg
                # self loop slot
                idxs[c, :, co][real] = ranks[real].astype(np.uint32)
                # edge slots
                lo = indptr[np.minimum(ranks, N - 1)]
                hi = indptr[np.minimum(ranks, N - 1) + 1]
                L = np.where(real, hi - lo, 0)
                kmax = int(L.max()) if L.size else 0
                for k in range(kmax):
                    sel = k < L
                    idxs[c, sel, co + 1 + k] = srcr_s[lo[sel] + k]
    return idxs


def kernel(x, edge_index, batch, W1, as1, ad1, b1, W2, as2, ad2, b2,
           W3, as3, ad3, b3, linW, linb):
    import jax

    x = np.asarray(x, np.float32)
    edge_index = np.asarray(edge_index)
    batch = np.asarray(batch)
    W1, W2, W3 = (np.asarray(w, np.float32) for w in (W1, W2, W3))
    as1, ad1, as2, ad2, as3, ad3 = (np.asarray(a, np.float32)
                                    for a in (as1, ad1, as2, ad2, as3, ad3))
    b1, b2, b3 = (np.asarray(b, np.float32) for b in (b1, b2, b3))
    linW = np.asarray(linW, np.float32)
    linb = np.asarray(linb, np.float32)

    N, F_in = x.shape
    src = edge_index[0]
    dst = edge_index[1]

    gp = _prep_graph(N, src, dst)
    R_TOT, order, rank = gp["R_TOT"], gp["order"], gp["rank"]
    T_core = gp["T_core"]
    shr = T_core * P

    key = (N, int(edge_index.shape[1]))
    if key not in _RUNNERS:
        g1, tc1 = _make_groups(gp["K_hat"], COLS_BUDGET[F_in + 2])
        g2, tc2 = _make_groups(gp["K_hat"], COLS_BUDGET[F_OUT + 2])
        idx1 = _fill_idx(gp, g1, tc1)
        idx2 = _fill_idx(gp, g2, tc2)
        nc1 = _build_layer_kernel(F_in + 2, R_TOT, g1, tc1, shr)
        fn1, _ = _make_runner(nc1, {"tbl", "w", "b"})
        nc2 = _build_layer_kernel(F_OUT + 2, R_TOT, g2, tc2, shr)
        fn2, _ = _make_runner(nc2, {"tbl", "w", "b"})
        from jax.sharding import Mesh, PartitionSpec, NamedSharding
        mesh = Mesh(np.asarray(jax.devices()[:N_CORES]), ("core",))
        sh = NamedSharding(mesh, PartitionSpec("core"))
        idx1g = jax.device_put(idx1.reshape(N_CORES * P, tc1), sh)
        idx2g = jax.device_put(idx2.reshape(N_CORES * P, tc2), sh)
        _RUNNERS[key] = (fn1, fn2, idx1g, idx2g)
    fn1, fn2, idx1g, idx2g = _RUNNERS[key]

    def build_table(prev, wa, wd, RC):
        """prev [N, K] by node -> table [R_TOT, RC] by rank."""
        K = prev.shape[1]
        t = np.zeros((R_TOT, RC), np.float32)
        t[rank[np.arange(N)], :K] = prev
        es = prev.astype(np.float64) @ wa.astype(np.float64)
        ed = prev.astype(np.float64) @ wd.astype(np.float64)
        t[rank[np.arange(N)], K] = es.astype(np.float32)
        t[rank[np.arange(N)], K + 1] = ed.astype(np.float32)
        t[gp["DUMMY"], K] = -200.0  # padding slots contribute exp(-inf)=0
        return t

    def unshard(o):
        """[8*shr, 64] -> by-node [N, 64]."""
        o = o.reshape(N_CORES, T_core, P, F_OUT)
        full = np.zeros((gp["n_tiles"] * P, F_OUT), np.float32)
        for c in range(N_CORES):
            for j in range(T_core):
                tile_id = j * N_CORES + c
                if tile_id < gp["n_tiles"]:
                    full[tile_id * P:(tile_id + 1) * P] = o[c, j]
        return full[rank[np.arange(N)]]

    import time
    times = []

    def run(fn, table, idxg, W, b):
        ins = {"tbl": table, "idx": idxg,
               "w": np.ascontiguousarray(W),
               "b": np.tile(b.reshape(1, F_OUT), (P, 1))}
        t0 = time.perf_counter()
        o = fn(ins)
        times.append(time.perf_counter() - t0)
        return unshard(o)

    t1 = build_table(x, W1 @ as1, W1 @ ad1, F_in + 2)
    out1 = run(fn1, t1, idx1g, W1, b1)
    prev2 = np.maximum(out1, 0.0)
    t2 = build_table(prev2, W2 @ as2, W2 @ ad2, F_OUT + 2)
    out2 = run(fn2, t2, idx2g, W2, b2)
    prev3 = np.maximum(out2, 0.0)
    t3 = build_table(prev3, W3 @ as3, W3 @ ad3, F_OUT + 2)
    h = run(fn2, t3, idx2g, W3, b3)

    kernel._launch_times = times

    # global mean+max pool by graph (batch sorted), then final linear
    G = 512
    b64 = np.asarray(batch).astype(np.int64)
    starts = np.searchsorted(b64, np.arange(G))
    ends = np.searchsorted(b64, np.arange(G), side="right")
    counts = (ends - starts).astype(np.float32)
    gmean = np.zeros((G, F_OUT), np.float32)
    gmax = np.zeros((G, F_OUT), np.float32)
    ne = counts > 0
    if ne.any():
        sums = np.add.reduceat(h, starts[ne], axis=0)
        gmean[ne] = sums / counts[ne, None]
        gmax[ne] = np.array([h[starts[g]:ends[g]].max(0)
                             for g in np.flatnonzero(ne)], np.float32)
    pooled = np.concatenate([gmean, gmax], axis=1)
    return (pooled @ linW + linb).astype(np.float32)



# revision 11
# speedup vs baseline: 18.8698x; 18.8698x over previous
"""GAT GNN kernel for 8 Trainium2 NeuronCores (Bass, via PJRT/axon).

Single fused launch for all three GAT layers. Strategy:
  - Nodes are permuted by in-degree so each 128-node dst tile has
    near-uniform degree -> tight ELL slot grids, shared by all layers.
  - Host computes h1 = x @ W1 and es/ed logit terms, ships per-core bf16
    table shards (row = [h(64) | es_hi | es_lo | ed_hi | ed_lo]) plus the
    ELL index grid (uint32 positions into the AllGathered table).
  - Device, per layer: AllGather table shards DRAM->DRAM across the 8
    cores (NeuronLink, not the host tunnel), per-tile indirect-DMA
    gathers, on-chip segment softmax (DVE+ACT), slot reduction, then one
    PE matmul per tile with rhs [W_next | W_next@a_src | W_next@a_dst]
    which directly yields the next layer's [h | es | ed] table shard.
  - Only the final [N, 64] bf16 node features return to host; mean/max
    pooling by graph and the final linear run on host.
Per-layer tables stay on device; host<->device traffic is ~28 MB up +
13 MB down total (the axon tunnel moves ~40 MB/s, so this dominates
wall time; device exec is ~10 ms, Pool-engine SWDGE bound).
"""
import sys

sys.path.insert(0, "/opt/trn_rl_repo")

import numpy as np

P = 128
F = 64                 # hidden width
RC = 68                # table row: 64 feats + es_hi, es_lo, ed_hi, ed_lo
RCO = 66               # matmul output row: 64 feats + es + ed (fp32)
NEG_SLOPE = 0.2
N_CORES = 8
COLS_BUDGET = 224      # gather cols per group
MAX_NT = 8
G = 512                # graphs

_RUNNERS = {}


def _bf16():
    import ml_dtypes
    return ml_dtypes.bfloat16


def _make_runner(nc, replicated_names):
    """jit the bass module over 8 cores via shard_map. Inputs in
    replicated_names get PartitionSpec(None); others shard on axis 0."""
    import jax
    from jax.sharding import Mesh, PartitionSpec
    from jax.experimental.shard_map import shard_map
    import concourse.mybir as mybir
    from concourse.bass2jax import (_bass_exec_p, partition_id_tensor,
                                    install_neuronx_cc_hook)

    install_neuronx_cc_hook()
    nc.finalize()
    partition_name = nc.partition_id_tensor.name if nc.partition_id_tensor else None

    in_names, out_names, out_avals, zero_outs = [], [], [], []
    for alloc in nc.m.functions[0].allocations:
        if not isinstance(alloc, mybir.MemoryLocationSet):
            continue
        name = alloc.memorylocations[0].name
        if alloc.kind == "ExternalInput":
            if name != partition_name:
                in_names.append(name)
        elif alloc.kind == "ExternalOutput":
            shape = tuple(alloc.tensor_shape)
            dtype = mybir.dt.np(alloc.dtype)
            out_names.append(name)
            out_avals.append(jax.core.ShapedArray(shape, dtype))
            zero_outs.append(np.zeros(shape, dtype))
    all_in = in_names + out_names + ([partition_name] if partition_name else [])

    def _body(*args):
        operands = list(args)
        if partition_name is not None:
            operands.append(partition_id_tensor())
        return tuple(_bass_exec_p.bind(
            *operands,
            out_avals=tuple(out_avals), in_names=tuple(all_in),
            out_names=tuple(out_names), lowering_input_output_aliases=(),
            sim_require_finite=False, sim_require_nnan=False, nc=nc))

    devices = jax.devices()[:N_CORES]
    mesh = Mesh(np.asarray(devices), ("core",))
    in_specs = tuple(
        PartitionSpec(None) if n in replicated_names else PartitionSpec("core")
        for n in in_names) + (PartitionSpec("core"),) * len(out_names)
    out_specs = (PartitionSpec("core"),) * len(out_names)
    jfn = jax.jit(shard_map(_body, mesh=mesh, in_specs=in_specs,
                            out_specs=out_specs, check_rep=False),
                  keep_unused=True)

    def fn(global_ins):
        import jax as _jax
        args = [global_ins[n] for n in in_names]
        args += [np.zeros((N_CORES * z.shape[0], *z.shape[1:]), z.dtype)
                 for z in zero_outs]
        outs = jfn(*args)
        _jax.block_until_ready(outs)
        return np.asarray(outs[0])

    return fn, in_names, jfn, replicated_names


def _build_fused_kernel(groups, TOTC, SHR, T_core):
    """All three GAT layers in one module, one core's dst shard each."""
    import concourse.bacc as bacc
    import concourse.bass as bass
    import concourse.mybir as mybir
    import concourse.tile as tile
    from concourse.masks import make_identity

    DT = mybir.dt.float32
    BF = mybir.dt.bfloat16
    A = mybir.AluOpType
    RTOT = N_CORES * SHR

    nc = bacc.Bacc("TRN2", target_bir_lowering=False, debug=False,
                   num_devices=N_CORES)
    t1_d = nc.dram_tensor("t1", [SHR, RC], BF, kind="ExternalInput")
    idx_d = nc.dram_tensor("idx", [P, TOTC], mybir.dt.uint32,
                           kind="ExternalInput")
    esb_d = nc.dram_tensor("esb", [P, T_core], DT, kind="ExternalInput")
    w2_d = nc.dram_tensor("w2", [F, RCO], DT, kind="ExternalInput")
    w3_d = nc.dram_tensor("w3", [F, RCO], DT, kind="ExternalInput")
    b1_d = nc.dram_tensor("b1", [P, F], DT, kind="ExternalInput")
    b2_d = nc.dram_tensor("b2", [P, F], DT, kind="ExternalInput")
    b3_d = nc.dram_tensor("b3", [P, F], DT, kind="ExternalInput")
    out_d = nc.dram_tensor("out", [SHR, F], BF, kind="ExternalOutput")

    with tile.TileContext(nc) as tc:
        with (tc.tile_pool(name="const", bufs=1) as cpool,
              tc.tile_pool(name="sb", bufs=2) as pool,
              tc.tile_pool(name="ps", bufs=2, space="PSUM") as pspool,
              tc.tile_pool(name="dS", bufs=2, space="DRAM") as dramS,
              tc.tile_pool(name="dT", bufs=2, space="DRAM") as dramT):
            ident = cpool.tile([P, P], DT)
            make_identity(nc, ident[:])
            w_sb = {}
            for lname, wd in ((2, w2_d), (3, w3_d)):
                w_sb[lname] = cpool.tile([F, RCO], DT, name=f"wsb{lname}")
                nc.sync.dma_start(out=w_sb[lname][:], in_=wd[:])
            b_sb = {}
            for lname, bd in ((1, b1_d), (2, b2_d), (3, b3_d)):
                b_sb[lname] = cpool.tile([P, F], DT, name=f"bsb{lname}")
                nc.sync.dma_start(out=b_sb[lname][:], in_=bd[:])
            esb_sb = cpool.tile([P, T_core], DT)
            nc.sync.dma_start(out=esb_sb[:], in_=esb_d[:])

            # layer-1 table: bounce the input shard, AllGather to full T
            S0 = dramS.tile([SHR, RC], BF, tag="S")
            nc.gpsimd.dma_start(out=S0[:], in_=t1_d[:])
            T_cur = dramT.tile([RTOT, RC], BF, tag="T")
            nc.gpsimd.collective_compute(
                "AllGather", A.bypass,
                replica_groups=[list(range(N_CORES))],
                ins=[S0.opt()], outs=[T_cur.opt()])

            def bcast(t, dims):
                b = t[:]
                return bass.AP(b.tensor, b.offset, [[b.ap[0][0], P]] + dims)

            for layer in (1, 2, 3):
                S_next = (dramS.tile([SHR, RC], BF, tag="S", name="Snext")
                          if layer < 3 else None)
                assert T_cur[:].offset == 0
                for (col_off, j0, nt, Kg) in groups:
                    cols = nt * Kg
                    it = pool.tile([P, cols], mybir.dt.uint32, tag="it")
                    nc.sync.dma_start(out=it[:],
                                      in_=idx_d[:, col_off:col_off + cols])
                    g = pool.tile([P, cols * RC], BF, tag="g")
                    for cc in range(cols):
                        nc.gpsimd.indirect_dma_start(
                            out=g[:, cc * RC:(cc + 1) * RC], out_offset=None,
                            in_=T_cur[:],
                            in_offset=bass.IndirectOffsetOnAxis(
                                ap=it[:, cc:cc + 1], axis=0))
                    gb = g[:]
                    pstep = gb.ap[0][0]

                    def gap(off, dims):
                        return bass.AP(gb.tensor, gb.offset + off,
                                       [[pstep, P]] + dims)

                    # z = es_hi + es_lo (slot) + ed_hi + ed_lo (self slot 0)
                    z = pool.tile([P, cols], DT, tag="z")
                    nc.vector.tensor_tensor(
                        out=z[:], in0=gap(F, [[RC, cols]]),
                        in1=gap(F + 1, [[RC, cols]]), op=A.add)
                    zed = pool.tile([P, nt], DT, tag="zed")
                    nc.vector.tensor_tensor(
                        out=zed[:], in0=gap(F + 2, [[Kg * RC, nt]]),
                        in1=gap(F + 3, [[Kg * RC, nt]]), op=A.add)
                    nc.vector.tensor_tensor(
                        out=z[:], in0=z[:],
                        in1=bcast(zed, [[1, nt], [0, Kg]]), op=A.add)
                    # leaky relu (exact): z = max(z, 0.2z), clamp at -30
                    zt = pool.tile([P, cols], DT, tag="zt")
                    nc.vector.tensor_scalar_mul(zt[:], z[:], NEG_SLOPE)
                    nc.vector.tensor_tensor(out=z[:], in0=z[:], in1=zt[:],
                                            op=A.max)
                    nc.vector.tensor_scalar_max(z[:], z[:], -30.0)
                    pex = pool.tile([P, cols], BF, tag="pex")
                    nc.scalar.activation(pex[:], z[:],
                                         mybir.ActivationFunctionType.Exp)
                    den = pool.tile([P, nt], DT, tag="den")
                    nc.vector.tensor_reduce(
                        out=den[:],
                        in_=pex[:].rearrange("p (t k) -> p t k", k=Kg),
                        axis=mybir.AxisListType.X, op=A.add)
                    nc.vector.reciprocal(den[:], den[:])
                    # feats *= p (in place, bf16)
                    nc.vector.tensor_tensor(
                        out=gap(0, [[RC, cols], [1, F]]),
                        in0=gap(0, [[RC, cols], [1, F]]),
                        in1=bcast(pex, [[1, cols], [0, F]]),
                        op=A.mult)
                    # reduce slots -> agg [P, nt*F] fp32
                    agg = pool.tile([P, nt * F], DT, tag="agg")
                    nc.vector.tensor_reduce(
                        out=agg[:],
                        in_=gap(0, [[Kg * RC, nt], [1, F], [RC, Kg]]),
                        axis=mybir.AxisListType.X, op=A.add)
                    nc.vector.tensor_tensor(
                        out=agg[:], in0=agg[:],
                        in1=bcast(den, [[1, nt], [0, F]]), op=A.mult)
                    nc.vector.tensor_tensor(
                        out=agg[:], in0=agg[:],
                        in1=bcast(b_sb[layer], [[0, nt], [1, F]]), op=A.add)

                    if layer == 3:
                        outf = pool.tile([P, nt * F], BF, tag="outf")
                        nc.vector.tensor_copy(out=outf[:], in_=agg[:])
                        ob = out_d[:]
                        dst = bass.AP(ob.tensor, ob.offset + j0 * P * F,
                                      [[F, P], [P * F, nt], [1, F]])
                        nc.sync.dma_start(out=dst, in_=outf[:])
                        continue

                    # relu, transpose, matmul -> [h | es | ed] fp32
                    nc.vector.tensor_scalar_max(agg[:], agg[:], 0.0)
                    aggv = agg[:].rearrange("p (t f) -> p t f", f=F)
                    psT = pspool.tile([F, nt * P], DT, tag="psT")
                    for t in range(nt):
                        nc.tensor.transpose(out=psT[:, t * P:(t + 1) * P],
                                            in_=aggv[:, t, :],
                                            identity=ident[:])
                    aggT = pool.tile([F, nt * P], DT, tag="aggT")
                    nc.vector.tensor_copy(out=aggT[:], in_=psT[:])
                    # 128-float (512B) stride per tile so no matmul output
                    # crosses a 2KB PSUM bank boundary
                    RCOP = 128
                    psO = pspool.tile([P, nt * RCOP], DT, tag="psO")
                    for t in range(nt):
                        nc.tensor.matmul(
                            out=psO[:, t * RCOP:t * RCOP + RCO],
                            lhsT=aggT[:, t * P:(t + 1) * P],
                            rhs=w_sb[layer + 1][:], start=True, stop=True)
                    pb = psO[:]
                    pstep2 = pb.ap[0][0]

                    def pap(off, dims):
                        return bass.AP(pb.tensor, pb.offset + off,
                                       [[pstep2, P]] + dims)

                    nxt = pool.tile([P, nt * RC], BF, tag="nxt")
                    nb = nxt[:]
                    nstep = nb.ap[0][0]

                    def nap(off, dims):
                        return bass.AP(nb.tensor, nb.offset + off,
                                       [[nstep, P]] + dims)

                    # feats
                    nc.vector.tensor_copy(
                        out=nap(0, [[RC, nt], [1, F]]),
                        in_=pap(0, [[RCOP, nt], [1, F]]))
                    # es_eff = es + bias (bias -200 on pad rows); hi/lo bf16
                    esum = pool.tile([P, nt], DT, tag="esum")
                    eb = esb_sb[:]
                    nc.vector.tensor_tensor(
                        out=esum[:], in0=pap(F, [[RCOP, nt]]),
                        in1=bass.AP(eb.tensor, eb.offset + j0,
                                    [[eb.ap[0][0], P], [1, nt]]),
                        op=A.add)
                    nc.vector.tensor_copy(out=nap(F, [[RC, nt]]),
                                          in_=esum[:])
                    hif = pool.tile([P, nt], DT, tag="hif")
                    nc.vector.tensor_copy(out=hif[:], in_=nap(F, [[RC, nt]]))
                    nc.vector.tensor_tensor(
                        out=nap(F + 1, [[RC, nt]]), in0=esum[:], in1=hif[:],
                        op=A.subtract)
                    # ed hi/lo
                    nc.vector.tensor_copy(out=nap(F + 2, [[RC, nt]]),
                                          in_=pap(F + 1, [[RCOP, nt]]))
                    hif2 = pool.tile([P, nt], DT, tag="hif2")
                    nc.vector.tensor_copy(out=hif2[:],
                                          in_=nap(F + 2, [[RC, nt]]))
                    nc.vector.tensor_tensor(
                        out=nap(F + 3, [[RC, nt]]),
                        in0=pap(F + 1, [[RCOP, nt]]), in1=hif2[:],
                        op=A.subtract)
                    sb = S_next[:]
                    dst = bass.AP(sb.tensor, sb.offset + j0 * P * RC,
                                  [[RC, P], [P * RC, nt], [1, RC]])
                    nc.sync.dma_start(out=dst, in_=nxt[:])

                if layer < 3:
                    T_cur = dramT.tile([RTOT, RC], BF, tag="T")
                    nc.gpsimd.collective_compute(
                        "AllGather", A.bypass,
                        replica_groups=[list(range(N_CORES))],
                        ins=[S_next.opt()], outs=[T_cur.opt()])
    return nc


def _prep_graph(N, src, dst):
    """Degree-permuted ELL layout shared by all layers."""
    deg = np.bincount(dst, minlength=N).astype(np.int64) + 1  # + self loop
    order = np.argsort(deg, kind="stable")
    rank = np.empty(N, np.int64)
    rank[order] = np.arange(N)
    n_tiles = (N + P - 1) // P
    T_core = (n_tiles + N_CORES - 1) // N_CORES
    SHR = T_core * P
    RTOT = N_CORES * SHR

    # rank r -> gathered-table position
    r_all = np.arange(RTOT, dtype=np.int64)
    t_all = r_all >> 7
    posr = (t_all % N_CORES) * SHR + (t_all // N_CORES) * P + (r_all & 127)
    DUMMY = np.uint32(posr[RTOT - 1])

    # CSR over dst ranks
    dstr = rank[dst]
    ord_e = np.argsort(dstr, kind="stable")
    dstr_s = dstr[ord_e]
    srcp_s = posr[rank[src][ord_e]].astype(np.uint32)
    indptr = np.searchsorted(dstr_s, np.arange(N + 1))
    k_e = np.arange(len(dstr_s), dtype=np.int64) - indptr[dstr_s]

    degp = np.zeros(RTOT, np.int64)
    degp[:N] = deg[order]
    tile_max = degp.reshape(-1, P).max(1)
    K_hat = np.maximum(
        2, tile_max.reshape(T_core, N_CORES).max(1))

    groups = []
    j = 0
    col = 0
    while j < T_core:
        nt = 1
        kg = int(K_hat[j])
        while (j + nt < T_core and nt < MAX_NT
               and (nt + 1) * max(kg, int(K_hat[j + nt])) <= COLS_BUDGET):
            kg = max(kg, int(K_hat[j + nt]))
            nt += 1
        groups.append((col, j, nt, kg))
        col += nt * kg
        j += nt
    TOTC = col

    col_base = np.empty(T_core, np.int64)
    for (col_off, j0, nt, kg) in groups:
        for t in range(nt):
            col_base[j0 + t] = col_off + t * kg

    # ELL grid [cores, P, TOTC]
    ell = np.full((N_CORES, P, TOTC), DUMMY, np.uint32)
    r = np.arange(N, dtype=np.int64)
    t = r >> 7
    ell[t % N_CORES, r & 127, col_base[t // N_CORES]] = posr[r].astype(
        np.uint32)  # self slots
    te = dstr_s >> 7
    ell[te % N_CORES, dstr_s & 127,
        col_base[te // N_CORES] + 1 + k_e] = srcp_s

    # es bias: -200 on pad rows (rank >= N)
    esb = np.zeros((N_CORES, P, T_core), np.float32)
    rp = np.arange(N, RTOT, dtype=np.int64)
    tp = rp >> 7
    esb[tp % N_CORES, rp & 127, tp // N_CORES] = -200.0

    return dict(order=order, rank=rank, posr=posr, T_core=T_core, SHR=SHR,
                RTOT=RTOT, groups=groups, TOTC=TOTC, ell=ell, esb=esb)


def _hi_lo(v, bf):
    hi = v.astype(bf)
    lo = (v - hi.astype(np.float32)).astype(bf)
    return hi, lo


def kernel(x, edge_index, batch, W1, as1, ad1, b1, W2, as2, ad2, b2,
           W3, as3, ad3, b3, linW, linb):
    import time
    bf = _bf16()

    x = np.asarray(x, np.float32)
    edge_index = np.asarray(edge_index)
    batch = np.asarray(batch).astype(np.int64)
    W1, W2, W3 = (np.asarray(w, np.float32) for w in (W1, W2, W3))
    as1, ad1, as2, ad2, as3, ad3 = (np.asarray(a, np.float32)
                                    for a in (as1, ad1, as2, ad2, as3, ad3))
    b1, b2, b3 = (np.asarray(b, np.float32) for b in (b1, b2, b3))
    linW = np.asarray(linW, np.float32)
    linb = np.asarray(linb, np.float32)

    N = x.shape[0]
    src = edge_index[0].astype(np.int64)
    dst = edge_index[1].astype(np.int64)

    t_prep0 = time.perf_counter()
    gp = _prep_graph(N, src, dst)
    SHR, T_core, RTOT = gp["SHR"], gp["T_core"], gp["RTOT"]
    posr, rank = gp["posr"], gp["rank"]
    kernel._prep_time = time.perf_counter() - t_prep0

    key = (N, int(edge_index.shape[1]), gp["TOTC"])
    if key not in _RUNNERS:
        nc = _build_fused_kernel(gp["groups"], gp["TOTC"], SHR, T_core)
        fn, in_names, jfn, repl = _make_runner(
            nc, {"w2", "w3", "b1", "b2", "b3"})
        _RUNNERS[key] = (fn, in_names)
    fn, in_names = _RUNNERS[key]

    # layer-1 table shard: [h1 | es_hi | es_lo | ed_hi | ed_lo] bf16 by pos
    t_host0 = time.perf_counter()
    h1 = x @ W1
    es1 = h1 @ as1
    ed1 = h1 @ ad1
    tbl = np.zeros((RTOT, RC), bf)
    tbl[:, F] = bf(-200.0)  # pad rows: es_eff = -200
    pos_n = posr[rank]      # node id -> table position
    tbl[pos_n, :F] = h1.astype(bf)
    eh, el = _hi_lo(es1, bf)
    tbl[pos_n, F] = eh
    tbl[pos_n, F + 1] = el
    dh, dl = _hi_lo(ed1, bf)
    tbl[pos_n, F + 2] = dh
    tbl[pos_n, F + 3] = dl

    w2m = np.concatenate([W2, (W2 @ as2)[:, None], (W2 @ ad2)[:, None]], 1)
    w3m = np.concatenate([W3, (W3 @ as3)[:, None], (W3 @ ad3)[:, None]], 1)
    ins = {
        "t1": tbl,  # [RTOT, RC] -> sharded to [SHR, RC] per core
        "idx": gp["ell"].reshape(N_CORES * P, gp["TOTC"]),
        "esb": gp["esb"].reshape(N_CORES * P, T_core),
        "w2": np.ascontiguousarray(w2m),
        "w3": np.ascontiguousarray(w3m),
        "b1": np.tile(b1.reshape(1, F), (P, 1)),
        "b2": np.tile(b2.reshape(1, F), (P, 1)),
        "b3": np.tile(b3.reshape(1, F), (P, 1)),
    }
    kernel._host_table_time = time.perf_counter() - t_host0

    t0 = time.perf_counter()
    out = fn(ins)
    kernel._launch_times = [time.perf_counter() - t0]

    t_post0 = time.perf_counter()
    h = out.astype(np.float32)[pos_n]  # [N, F] by node id

    # global mean+max pool by graph (batch sorted), then final linear
    starts = np.searchsorted(batch, np.arange(G))
    ends = np.searchsorted(batch, np.arange(G), side="right")
    counts = (ends - starts).astype(np.float32)
    gmean = np.zeros((G, F), np.float32)
    gmax = np.zeros((G, F), np.float32)
    ne = counts > 0
    if ne.any():
        sums = np.add.reduceat(h, starts[ne], axis=0)
        gmean[ne] = sums / counts[ne, None]
        gmax[ne] = np.maximum.reduceat(h, starts[ne], axis=0)
    pooled = np.concatenate([gmean, gmax], axis=1)
    kernel._post_time = time.perf_counter() - t_post0
    return (pooled @ linW + linb).astype(np.float32)


# revision 17
# speedup vs baseline: 1655.0901x; 87.7112x over previous
"""GAT GNN kernel for 8 Trainium2 NeuronCores (Bass, via PJRT/axon).

Single fused launch for all three GAT layers. Strategy:
  - Nodes are permuted by in-degree so each 128-node dst tile has
    near-uniform degree -> tight ELL slot grids, shared by all layers.
  - Host computes h1 = x @ W1 and es/ed logit terms, ships per-core bf16
    table shards (row = [h(64) | es_hi | es_lo | ed_hi | ed_lo]) plus the
    ELL index grid (uint32 positions into the AllGathered table).
  - Device, per layer: AllGather table shards DRAM->DRAM across the 8
    cores (NeuronLink, not the host tunnel), per-tile indirect-DMA
    gathers, on-chip segment softmax (DVE+ACT), slot reduction, then one
    PE matmul per tile with rhs [W_next | W_next@a_src | W_next@a_dst]
    which directly yields the next layer's [h | es | ed] table shard.
  - Only the final [N, 64] bf16 node features return to host; mean/max
    pooling by graph and the final linear run on host.
Per-layer tables stay on device; host<->device traffic is ~28 MB up +
13 MB down total (the axon tunnel moves ~40 MB/s, so this dominates
wall time; device exec is ~10 ms, Pool-engine SWDGE bound).
"""
import sys

sys.path.insert(0, "/opt/trn_rl_repo")

import numpy as np

P = 128
F = 64                 # hidden width
RC = 68                # table row: 64 feats + es_hi, es_lo, ed_hi, ed_lo
RCO = 66               # matmul output row: 64 feats + es + ed (fp32)
NEG_SLOPE = 0.2
N_CORES = 8
COLS_BUDGET = 224      # gather cols per group
MAX_NT = 8
G = 512                # graphs

_RUNNERS = {}


def _bf16():
    import ml_dtypes
    return ml_dtypes.bfloat16


def _make_runner(nc, replicated_names):
    """jit the bass module over 8 cores via shard_map. Inputs in
    replicated_names get PartitionSpec(None); others shard on axis 0."""
    import jax
    from jax.sharding import Mesh, PartitionSpec
    from jax.experimental.shard_map import shard_map
    import concourse.mybir as mybir
    from concourse.bass2jax import (_bass_exec_p, partition_id_tensor,
                                    install_neuronx_cc_hook)

    install_neuronx_cc_hook()
    nc.finalize()
    partition_name = nc.partition_id_tensor.name if nc.partition_id_tensor else None

    in_names, out_names, out_avals, zero_outs = [], [], [], []
    for alloc in nc.m.functions[0].allocations:
        if not isinstance(alloc, mybir.MemoryLocationSet):
            continue
        name = alloc.memorylocations[0].name
        if alloc.kind == "ExternalInput":
            if name != partition_name:
                in_names.append(name)
        elif alloc.kind == "ExternalOutput":
            shape = tuple(alloc.tensor_shape)
            dtype = mybir.dt.np(alloc.dtype)
            out_names.append(name)
            out_avals.append(jax.core.ShapedArray(shape, dtype))
            zero_outs.append(np.zeros(shape, dtype))
    all_in = in_names + out_names + ([partition_name] if partition_name else [])

    def _body(*args):
        operands = list(args)
        if partition_name is not None:
            operands.append(partition_id_tensor())
        return tuple(_bass_exec_p.bind(
            *operands,
            out_avals=tuple(out_avals), in_names=tuple(all_in),
            out_names=tuple(out_names), lowering_input_output_aliases=(),
            sim_require_finite=False, sim_require_nnan=False, nc=nc))

    devices = jax.devices()[:N_CORES]
    mesh = Mesh(np.asarray(devices), ("core",))
    in_specs = tuple(
        PartitionSpec(None) if n in replicated_names else PartitionSpec("core")
        for n in in_names) + (PartitionSpec("core"),) * len(out_names)
    out_specs = (PartitionSpec("core"),) * len(out_names)
    jfn = jax.jit(shard_map(_body, mesh=mesh, in_specs=in_specs,
                            out_specs=out_specs, check_rep=False),
                  keep_unused=True)

    dev_zeros = []

    def fn(global_ins):
        import jax as _jax
        from jax.sharding import NamedSharding
        args = [global_ins[n] for n in in_names]
        if not dev_zeros:
            # outputs are fully written by the kernel; keep one
            # device-resident zero buffer instead of re-uploading 12.8MB
            # of zeros through the tunnel every call
            sh = NamedSharding(mesh, PartitionSpec("core"))
            for z in zero_outs:
                dev_zeros.append(_jax.device_put(
                    np.zeros((N_CORES * z.shape[0], *z.shape[1:]), z.dtype),
                    sh))
        args += dev_zeros
        outs = jfn(*args)
        _jax.block_until_ready(outs)
        return np.asarray(outs[0])

    return fn, in_names, jfn, replicated_names


def _build_fused_kernel(groups, TOTC, SHR, T_core):
    """All three GAT layers in one module, one core's dst shard each."""
    import concourse.bacc as bacc
    import concourse.bass as bass
    import concourse.mybir as mybir
    import concourse.tile as tile
    from concourse.masks import make_identity

    DT = mybir.dt.float32
    BF = mybir.dt.bfloat16
    A = mybir.AluOpType
    RTOT = N_CORES * SHR

    nc = bacc.Bacc("TRN2", target_bir_lowering=False, debug=False,
                   num_devices=N_CORES)
    t1_d = nc.dram_tensor("t1", [SHR, RC], BF, kind="ExternalInput")
    idx_d = nc.dram_tensor("idx", [P, TOTC], mybir.dt.uint32,
                           kind="ExternalInput")
    esb_d = nc.dram_tensor("esb", [P, T_core], DT, kind="ExternalInput")
    w2_d = nc.dram_tensor("w2", [F, RCO], DT, kind="ExternalInput")
    w3_d = nc.dram_tensor("w3", [F, RCO], DT, kind="ExternalInput")
    b1_d = nc.dram_tensor("b1", [P, F], DT, kind="ExternalInput")
    b2_d = nc.dram_tensor("b2", [P, F], DT, kind="ExternalInput")
    b3_d = nc.dram_tensor("b3", [P, F], DT, kind="ExternalInput")
    out_d = nc.dram_tensor("out", [SHR, F], BF, kind="ExternalOutput")

    with tile.TileContext(nc) as tc:
        with (tc.tile_pool(name="const", bufs=1) as cpool,
              tc.tile_pool(name="sb", bufs=2) as pool,
              tc.tile_pool(name="ps", bufs=2, space="PSUM") as pspool,
              tc.tile_pool(name="dS", bufs=2, space="DRAM") as dramS,
              tc.tile_pool(name="dT", bufs=2, space="DRAM") as dramT):
            ident = cpool.tile([P, P], DT)
            make_identity(nc, ident[:])
            w_sb = {}
            for lname, wd in ((2, w2_d), (3, w3_d)):
                w_sb[lname] = cpool.tile([F, RCO], DT, name=f"wsb{lname}")
                nc.sync.dma_start(out=w_sb[lname][:], in_=wd[:])
            b_sb = {}
            for lname, bd in ((1, b1_d), (2, b2_d), (3, b3_d)):
                b_sb[lname] = cpool.tile([P, F], DT, name=f"bsb{lname}")
                nc.sync.dma_start(out=b_sb[lname][:], in_=bd[:])
            esb_sb = cpool.tile([P, T_core], DT)
            nc.sync.dma_start(out=esb_sb[:], in_=esb_d[:])

            # layer-1 table: bounce the input shard, AllGather to full T
            S0 = dramS.tile([SHR, RC], BF, tag="S")
            nc.gpsimd.dma_start(out=S0[:], in_=t1_d[:])
            T_cur = dramT.tile([RTOT, RC], BF, tag="T")
            nc.gpsimd.collective_compute(
                "AllGather", A.bypass,
                replica_groups=[list(range(N_CORES))],
                ins=[S0.opt()], outs=[T_cur.opt()])

            def bcast(t, dims):
                b = t[:]
                return bass.AP(b.tensor, b.offset, [[b.ap[0][0], P]] + dims)

            for layer in (1, 2, 3):
                S_next = (dramS.tile([SHR, RC], BF, tag="S", name="Snext")
                          if layer < 3 else None)
                assert T_cur[:].offset == 0
                for (col_off, j0, nt, Kg) in groups:
                    cols = nt * Kg
                    it = pool.tile([P, cols], mybir.dt.uint32, tag="it")
                    nc.sync.dma_start(out=it[:],
                                      in_=idx_d[:, col_off:col_off + cols])
                    g = pool.tile([P, cols * RC], BF, tag="g")
                    for cc in range(cols):
                        nc.gpsimd.indirect_dma_start(
                            out=g[:, cc * RC:(cc + 1) * RC], out_offset=None,
                            in_=T_cur[:],
                            in_offset=bass.IndirectOffsetOnAxis(
                                ap=it[:, cc:cc + 1], axis=0))
                    gb = g[:]
                    pstep = gb.ap[0][0]

                    def gap(off, dims):
                        return bass.AP(gb.tensor, gb.offset + off,
                                       [[pstep, P]] + dims)

                    # z = es_hi + es_lo (slot) + ed_hi + ed_lo (self slot 0)
                    z = pool.tile([P, cols], DT, tag="z")
                    nc.vector.tensor_tensor(
                        out=z[:], in0=gap(F, [[RC, cols]]),
                        in1=gap(F + 1, [[RC, cols]]), op=A.add)
                    zed = pool.tile([P, nt], DT, tag="zed")
                    nc.vector.tensor_tensor(
                        out=zed[:], in0=gap(F + 2, [[Kg * RC, nt]]),
                        in1=gap(F + 3, [[Kg * RC, nt]]), op=A.add)
                    nc.vector.tensor_tensor(
                        out=z[:], in0=z[:],
                        in1=bcast(zed, [[1, nt], [0, Kg]]), op=A.add)
                    # leaky relu (exact): z = max(z, 0.2z), clamp at -30
                    zt = pool.tile([P, cols], DT, tag="zt")
                    nc.vector.tensor_scalar_mul(zt[:], z[:], NEG_SLOPE)
                    nc.vector.tensor_tensor(out=z[:], in0=z[:], in1=zt[:],
                                            op=A.max)
                    nc.vector.tensor_scalar_max(z[:], z[:], -30.0)
                    pex = pool.tile([P, cols], BF, tag="pex")
                    nc.scalar.activation(pex[:], z[:],
                                         mybir.ActivationFunctionType.Exp)
                    den = pool.tile([P, nt], DT, tag="den")
                    nc.vector.tensor_reduce(
                        out=den[:],
                        in_=pex[:].rearrange("p (t k) -> p t k", k=Kg),
                        axis=mybir.AxisListType.X, op=A.add)
                    nc.vector.reciprocal(den[:], den[:])
                    # feats *= p (in place, bf16)
                    nc.vector.tensor_tensor(
                        out=gap(0, [[RC, cols], [1, F]]),
                        in0=gap(0, [[RC, cols], [1, F]]),
                        in1=bcast(pex, [[1, cols], [0, F]]),
                        op=A.mult)
                    # reduce slots -> agg [P, nt*F] fp32
                    agg = pool.tile([P, nt * F], DT, tag="agg")
                    nc.vector.tensor_reduce(
                        out=agg[:],
                        in_=gap(0, [[Kg * RC, nt], [1, F], [RC, Kg]]),
                        axis=mybir.AxisListType.X, op=A.add)
                    nc.vector.tensor_tensor(
                        out=agg[:], in0=agg[:],
                        in1=bcast(den, [[1, nt], [0, F]]), op=A.mult)
                    nc.vector.tensor_tensor(
                        out=agg[:], in0=agg[:],
                        in1=bcast(b_sb[layer], [[0, nt], [1, F]]), op=A.add)

                    if layer == 3:
                        outf = pool.tile([P, nt * F], BF, tag="outf")
                        nc.vector.tensor_copy(out=outf[:], in_=agg[:])
                        ob = out_d[:]
                        dst = bass.AP(ob.tensor, ob.offset + j0 * P * F,
                                      [[F, P], [P * F, nt], [1, F]])
                        nc.sync.dma_start(out=dst, in_=outf[:])
                        continue

                    # relu, transpose, matmul -> [h | es | ed] fp32
                    nc.vector.tensor_scalar_max(agg[:], agg[:], 0.0)
                    aggv = agg[:].rearrange("p (t f) -> p t f", f=F)
                    psT = pspool.tile([F, nt * P], DT, tag="psT")
                    for t in range(nt):
                        nc.tensor.transpose(out=psT[:, t * P:(t + 1) * P],
                                            in_=aggv[:, t, :],
                                            identity=ident[:])
                    aggT = pool.tile([F, nt * P], DT, tag="aggT")
                    nc.vector.tensor_copy(out=aggT[:], in_=psT[:])
                    # 128-float (512B) stride per tile so no matmul output
                    # crosses a 2KB PSUM bank boundary
                    RCOP = 128
                    psO = pspool.tile([P, nt * RCOP], DT, tag="psO")
                    for t in range(nt):
                        nc.tensor.matmul(
                            out=psO[:, t * RCOP:t * RCOP + RCO],
                            lhsT=aggT[:, t * P:(t + 1) * P],
                            rhs=w_sb[layer + 1][:], start=True, stop=True)
                    pb = psO[:]
                    pstep2 = pb.ap[0][0]

                    def pap(off, dims):
                        return bass.AP(pb.tensor, pb.offset + off,
                                       [[pstep2, P]] + dims)

                    nxt = pool.tile([P, nt * RC], BF, tag="nxt")
                    nb = nxt[:]
                    nstep = nb.ap[0][0]

                    def nap(off, dims):
                        return bass.AP(nb.tensor, nb.offset + off,
                                       [[nstep, P]] + dims)

                    # feats
                    nc.vector.tensor_copy(
                        out=nap(0, [[RC, nt], [1, F]]),
                        in_=pap(0, [[RCOP, nt], [1, F]]))
                    # es_eff = es + bias (bias -200 on pad rows); hi/lo bf16
                    esum = pool.tile([P, nt], DT, tag="esum")
                    eb = esb_sb[:]
                    nc.vector.tensor_tensor(
                        out=esum[:], in0=pap(F, [[RCOP, nt]]),
                        in1=bass.AP(eb.tensor, eb.offset + j0,
                                    [[eb.ap[0][0], P], [1, nt]]),
                        op=A.add)
                    nc.vector.tensor_copy(out=nap(F, [[RC, nt]]),
                                          in_=esum[:])
                    hif = pool.tile([P, nt], DT, tag="hif")
                    nc.vector.tensor_copy(out=hif[:], in_=nap(F, [[RC, nt]]))
                    nc.vector.tensor_tensor(
                        out=nap(F + 1, [[RC, nt]]), in0=esum[:], in1=hif[:],
                        op=A.subtract)
                    # ed hi/lo
                    nc.vector.tensor_copy(out=nap(F + 2, [[RC, nt]]),
                                          in_=pap(F + 1, [[RCOP, nt]]))
                    hif2 = pool.tile([P, nt], DT, tag="hif2")
                    nc.vector.tensor_copy(out=hif2[:],
                                          in_=nap(F + 2, [[RC, nt]]))
                    nc.vector.tensor_tensor(
                        out=nap(F + 3, [[RC, nt]]),
                        in0=pap(F + 1, [[RCOP, nt]]), in1=hif2[:],
                        op=A.subtract)
                    sb = S_next[:]
                    dst = bass.AP(sb.tensor, sb.offset + j0 * P * RC,
                                  [[RC, P], [P * RC, nt], [1, RC]])
                    nc.sync.dma_start(out=dst, in_=nxt[:])

                if layer < 3:
                    T_cur = dramT.tile([RTOT, RC], BF, tag="T")
                    nc.gpsimd.collective_compute(
                        "AllGather", A.bypass,
                        replica_groups=[list(range(N_CORES))],
                        ins=[S_next.opt()], outs=[T_cur.opt()])
    return nc


def _prep_light(N, dst):
    """Degree permutation, table positions, ELL group geometry, es bias."""
    deg = np.bincount(dst, minlength=N).astype(np.int32) + 1  # + self loop
    order = np.argsort(deg, kind="stable")
    rank = np.empty(N, np.int32)
    rank[order] = np.arange(N, dtype=np.int32)
    n_tiles = (N + P - 1) // P
    T_core = (n_tiles + N_CORES - 1) // N_CORES
    SHR = T_core * P
    RTOT = N_CORES * SHR

    # rank r -> gathered-table position
    r_all = np.arange(RTOT, dtype=np.int32)
    t_all = r_all >> 7
    posr = ((t_all % N_CORES) * SHR + (t_all // N_CORES) * P
            + (r_all & 127)).astype(np.int32)
    DUMMY = np.uint32(posr[RTOT - 1])

    degp = np.zeros(RTOT, np.int32)
    degp[:N] = deg[order]
    tile_max = degp.reshape(-1, P).max(1)
    K_hat = np.maximum(2, tile_max.reshape(T_core, N_CORES).max(1))

    groups = []
    j = 0
    col = 0
    while j < T_core:
        nt = 1
        kg = int(K_hat[j])
        while (j + nt < T_core and nt < MAX_NT
               and (nt + 1) * max(kg, int(K_hat[j + nt])) <= COLS_BUDGET):
            kg = max(kg, int(K_hat[j + nt]))
            nt += 1
        groups.append((col, j, nt, kg))
        col += nt * kg
        j += nt
    TOTC = col

    col_base = np.empty(T_core, np.int64)
    for (col_off, j0, nt, kg) in groups:
        for t in range(nt):
            col_base[j0 + t] = col_off + t * kg

    # es bias: -200 on pad rows (rank >= N)
    esb = np.zeros((N_CORES, P, T_core), np.float32)
    rp = np.arange(N, RTOT, dtype=np.int64)
    tp = rp >> 7
    esb[tp % N_CORES, rp & 127, tp // N_CORES] = -200.0

    return dict(deg=deg, order=order, rank=rank, posr=posr, T_core=T_core,
                SHR=SHR, RTOT=RTOT, groups=groups, TOTC=TOTC,
                col_base=col_base, DUMMY=DUMMY, esb=esb)


def _prep_edges(gp, N, src, dst):
    """ELL slot grid fill [cores, P, TOTC] (order within a row is free,
    so an unstable int32 argsort suffices)."""
    rank, posr, col_base = gp["rank"], gp["posr"], gp["col_base"]
    TOTC, DUMMY = gp["TOTC"], gp["DUMMY"]
    posn = posr[rank]                       # node id -> table position
    dstr = rank[dst]                        # int32
    srcp = posn[src].astype(np.uint32)
    ord_e = np.argsort(dstr)                # unstable, fast
    dstr_s = dstr[ord_e]
    srcp_s = srcp[ord_e]
    cnt = np.bincount(dstr_s, minlength=N)
    indptr = np.zeros(N + 1, np.int64)
    np.cumsum(cnt, out=indptr[1:])
    k_e = np.arange(len(dstr_s), dtype=np.int64) - indptr[dstr_s]

    ell = np.full((N_CORES, P, TOTC), DUMMY, np.uint32)
    r = np.arange(N, dtype=np.int64)
    t = r >> 7
    ell[t % N_CORES, r & 127, col_base[t // N_CORES]] = \
        posr[:N].astype(np.uint32)  # self slot of rank r
    te = dstr_s >> 7
    ell[te % N_CORES, dstr_s & 127,
        col_base[te // N_CORES] + 1 + k_e] = srcp_s
    return ell


def _prep_graph(N, src, dst):
    """Compatibility wrapper: light + edge prep merged."""
    gp = _prep_light(N, dst)
    gp["ell"] = _prep_edges(gp, N, src, dst)
    return gp


def _hi_lo(v, bf):
    hi = v.astype(bf)
    lo = (v - hi.astype(np.float32)).astype(bf)
    return hi, lo


def kernel(x, edge_index, batch, W1, as1, ad1, b1, W2, as2, ad2, b2,
           W3, as3, ad3, b3, linW, linb):
    import time
    bf = _bf16()

    x = np.asarray(x, np.float32)
    edge_index = np.asarray(edge_index)
    batch = np.asarray(batch).astype(np.int64)
    W1, W2, W3 = (np.asarray(w, np.float32) for w in (W1, W2, W3))
    as1, ad1, as2, ad2, as3, ad3 = (np.asarray(a, np.float32)
                                    for a in (as1, ad1, as2, ad2, as3, ad3))
    b1, b2, b3 = (np.asarray(b, np.float32) for b in (b1, b2, b3))
    linW = np.asarray(linW, np.float32)
    linb = np.asarray(linb, np.float32)

    N = x.shape[0]
    src = edge_index[0].astype(np.int64)
    dst = edge_index[1].astype(np.int64)

    t_prep0 = time.perf_counter()
    gp = _prep_light(N, dst)
    SHR, T_core, RTOT = gp["SHR"], gp["T_core"], gp["RTOT"]
    posr, rank = gp["posr"], gp["rank"]

    key = (N, int(edge_index.shape[1]), gp["TOTC"])
    if key not in _RUNNERS:
        nc = _build_fused_kernel(gp["groups"], gp["TOTC"], SHR, T_core)
        fn, in_names, jfn, repl = _make_runner(
            nc, {"w2", "w3", "b1", "b2", "b3"})
        _RUNNERS[key] = (fn, in_names, jfn, repl)
    fn, in_names, _, _ = _RUNNERS[key]

    import jax
    from jax.sharding import Mesh, PartitionSpec, NamedSharding
    mesh = Mesh(np.asarray(jax.devices()[:N_CORES]), ("core",))
    shard = NamedSharding(mesh, PartitionSpec("core"))

    # layer-1 table shard: [h1 | es_hi | es_lo | ed_hi | ed_lo] bf16 by pos
    h1 = x @ W1
    es1 = h1 @ as1
    ed1 = h1 @ ad1
    tbl = np.zeros((RTOT, RC), bf)
    tbl[:, F] = bf(-200.0)  # pad rows: es_eff = -200
    pos_n = posr[rank]      # node id -> table position
    tbl[pos_n, :F] = h1.astype(bf)
    eh, el = _hi_lo(es1, bf)
    tbl[pos_n, F] = eh
    tbl[pos_n, F + 1] = el
    dh, dl = _hi_lo(ed1, bf)
    tbl[pos_n, F + 2] = dh
    tbl[pos_n, F + 3] = dl

    # start the 13.6MB table upload while the edge prep runs
    from concurrent.futures import ThreadPoolExecutor
    pool = ThreadPoolExecutor(2)
    fut_t1 = pool.submit(lambda: jax.device_put(tbl, shard))

    ell = _prep_edges(gp, N, src, dst)
    fut_idx = pool.submit(
        lambda: jax.device_put(ell.reshape(N_CORES * P, gp["TOTC"]), shard))
    kernel._prep_time = time.perf_counter() - t_prep0

    t_host0 = time.perf_counter()
    w2m = np.concatenate([W2, (W2 @ as2)[:, None], (W2 @ ad2)[:, None]], 1)
    w3m = np.concatenate([W3, (W3 @ as3)[:, None], (W3 @ ad3)[:, None]], 1)
    ins = {
        "t1": fut_t1.result(),
        "idx": fut_idx.result(),
        "esb": gp["esb"].reshape(N_CORES * P, T_core),
        "w2": np.ascontiguousarray(w2m),
        "w3": np.ascontiguousarray(w3m),
        "b1": np.tile(b1.reshape(1, F), (P, 1)),
        "b2": np.tile(b2.reshape(1, F), (P, 1)),
        "b3": np.tile(b3.reshape(1, F), (P, 1)),
    }
    pool.shutdown(wait=False)
    kernel._host_table_time = time.perf_counter() - t_host0
    kernel._last_ins = ins

    t0 = time.perf_counter()
    out = fn(ins)
    kernel._launch_times = [time.perf_counter() - t0]

    t_post0 = time.perf_counter()
    h = out.astype(np.float32)[pos_n]  # [N, F] by node id

    # global mean+max pool by graph (batch sorted), then final linear
    starts = np.searchsorted(batch, np.arange(G))
    ends = np.searchsorted(batch, np.arange(G), side="right")
    counts = (ends - starts).astype(np.float32)
    gmean = np.zeros((G, F), np.float32)
    gmax = np.zeros((G, F), np.float32)
    ne = counts > 0
    if ne.any():
        sums = np.add.reduceat(h, starts[ne], axis=0)
        gmean[ne] = sums / counts[ne, None]
        gmax[ne] = np.maximum.reduceat(h, starts[ne], axis=0)
    pooled = np.concatenate([gmean, gmax], axis=1)
    kernel._post_time = time.perf_counter() - t_post0
    return (pooled @ linW + linb).astype(np.float32)
